# revision 29
# baseline (speedup 1.0000x reference)
"""Multi-head attention (B=4, N=2048, C=1024, H=16) on 8 Trainium2 NeuronCores.

Sharding (per spec hint): data-parallel on batch, tensor-parallel on heads.
Core c -> (batch b = c//2, head-group hg = c%2 of 8 heads / 512 features).
Each core computes q/k/v for its 8 heads over the full 2048-token sequence,
full attention for those heads, and a PARTIAL output projection over its 512
features. The host sums the two partials per batch (the "all-reduce after
proj" done on host during unsharding).

Key structure (all matmuls bf16, fp32 PSUM):
- scores per (head-PAIR, query-half-of-512, key-block): psum [128 keys,
  1024] = [headA 512q | headB 512q], written by TWO K=64 matmuls issued
  back-to-back at PE row-tile positions (0,0) and (64,0) so the hardware
  runs them CONCURRENTLY on the two 64-row halves of the PE array (2x
  score throughput vs one head at a time; tile_position auto-derives
  from the kt/qt base partitions). exp on Act engine -> et bf16 in SBUF.
- attn@V "orientation B": stationary et-block [128k x 128q], moving
  v-pack [128k, 65] (64 features + ones column) accumulated over 16 key
  blocks -> psum [128q, 65] = unnormalized numerator + denominator.
  2x fewer PE cycles than streaming queries past stationary V.
- normalize: reciprocal of the denominator column + tensor_scalar_mul
  (per-partition scalar) -> attn_out halves of a shared [128q, 128] pair
  tile; ONE PE transpose per head pair -> attnT [128f, 128q] for the
  projection (half the transposes + full-partition DVE copies).
- K bias is skipped entirely (per-query-constant score shifts are softmax
  invariant), V bias is folded into the proj bias on the host.
- Software pipelining over 16 "stripes" (head-pair f, nb, query-half):
  scores/exp of stripe st overlap attn@V of stripe st-1; q/k/v projection
  matmuls are interleaved as PE filler spread by per-tile deadline (v is
  emitted in 2-head groups so its cost spreads over stripes 0-12 instead
  of jamming stripes 0-1); the last head pair runs nb=1 before nb=0 so
  the tail after the final exp is only 8 attn@V units + 4 staged proj
  blocks; the other 12 proj blocks run inside stripes 14-15.
"""

import sys
from contextlib import ExitStack

sys.path.insert(0, "/opt/trn_rl_repo")

import numpy as np
import ml_dtypes

import concourse.bacc as bacc
import concourse.mybir as mybir
import concourse.tile as tile
from concourse.bass_utils import run_bass_kernel_spmd

import concourse.dve_ops as dve_ops_mod
from concourse.dve_spec import C0, C1, C2, One, Spec, Src0, sq
from concourse.dve_uop import DveOpSpec


def _register_dve_op(name, spec):
    """Append a custom DVE op to the registry (the sanctioned extension path:
    new rows go at the end; sha computed from the lowered uops)."""
    D = dve_ops_mod
    for o in D.OPS:
        if o.name == name:
            return o
    row = D._CUSTOM_DVE_ROW_BASE + len(D.OPS)
    D._SUB_OPCODE_FOR_NAME[name] = row
    shas = {}
    for ver in ("v3", "v4"):
        uops = D.lower(spec, ver=ver)
        shas[ver] = DveOpSpec(
            name=name, opcode=row, uops=uops, rd1_en=D.has_src1(spec)
        ).sha(ver)
    op = D.DveOp(name, spec, subdim=False, uops_sha=shas)
    D.OPS.append(op)
    D.CUSTOM_DVE_SPECS[name] = spec
    return op


# exp(x) ~= (1 + y + y^2*(1/2 + y/6))^32 with y = x/32: cubic Taylor of
# e^(x/32) then 5 squarings. Rel err < 3e-4 for x in [-4, 5] (weights that
# matter); lets the (otherwise idle-ish) DVE take a slice of the softmax
# exp work off the saturated Act engine. Two DVE instructions per tile.
_Y = Src0 * C0
_EXP32_CUBIC = _register_dve_op(
    "EXP32_CUBIC_ANT",
    Spec(
        body=sq(_Y) * (_Y * C1 + C2) + _Y + One,
        reference=lambda in0, in1, s0, s1, imm2: (
            (in0 * s0) ** 2 * ((in0 * s0) * s1 + imm2) + (in0 * s0) + 1.0
        ),
    ),
)
_POW32 = _register_dve_op(
    "POW32_ANT",
    Spec(
        body=sq(sq(sq(sq(sq(Src0))))),
        reference=lambda in0, in1, s0, s1, imm2: in0**32,
    ),
)

B, N, C, H, D = 4, 2048, 1024, 16, 64
NCORES = 8
HPC = 8           # heads per core
FPC = HPC * D     # features per core (512)
NFTG = FPC // 128  # feature tile groups (4)
NMC = N // 128    # key blocks (16)
NTB = N // 512    # token blocks for 512-streams (4)
NTT = N // 128    # token blocks of 128 (16)
SCALE = float(D) ** -0.5
# exp offload to DVE only in the late stripes (>= 11), where the qkv PE
# filler has run out and the PE otherwise idles waiting on the Act engine
DVE_ST0 = 11
DVE_MC = (4, 9, 14)
BF16 = mybir.dt.bfloat16
F32 = mybir.dt.float32
AF = mybir.ActivationFunctionType
NPBF = ml_dtypes.bfloat16


def build_nc(reps=1):
    nc = bacc.Bacc("TRN2", target_bir_lowering=False, debug=False, num_devices=NCORES)

    xT = nc.dram_tensor("xT", [128, 8, N], BF16, kind="ExternalInput")
    wq = nc.dram_tensor("wq", [128, 8, FPC], BF16, kind="ExternalInput")
    wk = nc.dram_tensor("wk", [128, 8, FPC], BF16, kind="ExternalInput")
    wv = nc.dram_tensor("wv", [128, 8, FPC], BF16, kind="ExternalInput")
    wp = nc.dram_tensor("wp", [128, NFTG, C], BF16, kind="ExternalInput")
    bq = nc.dram_tensor("bq", [FPC], F32, kind="ExternalInput")
    bp = nc.dram_tensor("bp", [1, C], BF16, kind="ExternalInput")
    ident_in = nc.dram_tensor("ident_in", [128, 128], BF16, kind="ExternalInput")
    out = nc.dram_tensor("out", [N, C], F32, kind="ExternalOutput")

    with tile.TileContext(nc) as tc, ExitStack() as ctx:
        def P(name, bufs, space="SBUF"):
            return ctx.enter_context(tc.tile_pool(name=name, bufs=bufs, space=space))

        cst_p = P("cst", 1)
        xt_p = P("xt", 8)
        wqk_p = P("wqk", 4)
        wvp_p = P("wvp", 1)
        wpp_p = P("wpp", 1)
        qt_p = P("qt", 4)
        kt_p = P("kt", 4)
        vp_p = P("vp", 16)
        et_p = P("et", 30)
        ex_p = P("ex", 2)
        exs_p = P("exs", 1)
        ao_p = P("ao", 3)
        rc_p = P("rc", 3)
        at_p = P("at", 4)
        ot_p = P("ot", 2)
        mm_p = P("mm", 2, space="PSUM")   # qkv psums + transpose dests
        sc_p = P("sc", 2, space="PSUM")   # scores + proj psums  [128,1024]
        av_p = P("av", 2, space="PSUM")   # attn@V accumulators  [128,65]

        # ---- constants ----
        ident = cst_p.tile([128, 128], BF16, tag="ident")
        nc.sync.dma_start(ident[:], ident_in[:, :])
        ones_f = cst_p.tile([1, 128], F32, tag="ones_f")
        nc.gpsimd.memset(ones_f[:], 1.0)
        ones1 = cst_p.tile([1, 128], BF16, tag="ones1")
        nc.vector.tensor_copy(ones1[:], ones_f[:])
        onesc_f = cst_p.tile([128, HPC], F32, tag="onesc_f")
        nc.gpsimd.memset(onesc_f[:], 1.0)
        bqt = cst_p.tile([128, NFTG], F32, tag="bqt")
        nc.sync.dma_start(bqt[:], bq[:].rearrange("(a p) -> p a", p=128))
        bpt = cst_p.tile([1, C], BF16, tag="bpt")
        nc.sync.dma_start(bpt[:], bp[:, :])
        # proj bias broadcast to all partitions (built once): lets the proj
        # psum->sbuf copy fuse the bias add on DVE instead of 32 PE matmuls
        bb = cst_p.tile([128, C], BF16, tag="bb")
        ps0 = sc_p.tile([128, 1024], F32, tag="sc", name="bbinit")
        for cb in range(2):
            nc.tensor.matmul(
                ps0[:, cb * 512 : (cb + 1) * 512],
                ones1[0:1, 0:128],
                bpt[0:1, cb * 512 : (cb + 1) * 512],
                start=True,
                stop=True,
            )
        nc.vector.tensor_copy(bb[:], ps0[:])
        # proj bias broadcast to all partitions (built once): lets the proj
        # psum->sbuf copy fuse the bias add on DVE instead of 32 PE matmuls

        def rep_body():
            # ---- resident inputs ----
            # xt DMAs in tb-major order so the first k/q matmuls can start
            # after ~1MB instead of the full 4MB.
            xt = [xt_p.tile([128, N], BF16, tag="xt", name=f"xt{i}") for i in range(8)]

            def dma_xt_tb0():
                for cc in range(8):
                    nc.sync.dma_start(
                        xt[cc][:, 0:512], xT[:, cc, 0:512]
                    )

            def dma_xt_rest():
                # tb1 first as its own batch: phase-0 V/K/Q fillers touching
                # tokens 512..1024 unblock without waiting for the full 3MB
                for cc in range(8):
                    nc.sync.dma_start(
                        xt[cc][:, 512:1024], xT[:, cc, 512:1024]
                    )
                for cc in range(8):
                    nc.sync.dma_start(
                        xt[cc][:, 1024:N], xT[:, cc, 1024:N]
                    )

            wv_t = wvp_p.tile([128, 8 * FPC], BF16, tag="wv")
            wp_t = wpp_p.tile([128, NFTG * C], BF16, tag="wp")

            qt = [qt_p.tile([128, N], BF16, tag="qt", name=f"qt{i}") for i in range(NFTG)]
            kt = [kt_p.tile([128, N], BF16, tag="kt", name=f"kt{i}") for i in range(NFTG)]
            attnT = [
                at_p.tile([128, N], BF16, tag="at", name=f"at{i}") for i in range(NFTG)
            ]
            vp_tiles = [None] * NTT

            def load_wqk(kind, ftg):
                src = wq if kind == "q" else wk
                w = wqk_p.tile([128, 8 * 128], BF16, tag="wqk", name=f"w{kind}{ftg}")
                nc.sync.dma_start(
                    w[:].rearrange("p (a b) -> p a b", b=128),
                    src[:, :, ftg * 128 : (ftg + 1) * 128],
                )
                return w

            def emit_q_tb(ftg, tb, w):
                ps = mm_p.tile([128, 512], F32, tag="mm")
                for cc in range(8):
                    nc.tensor.matmul(
                        ps[:],
                        w[:, cc * 128 : (cc + 1) * 128],
                        xt[cc][:, tb * 512 : (tb + 1) * 512],
                        start=(cc == 0),
                        stop=(cc == 7),
                    )
                nc.vector.tensor_scalar_add(
                    qt[ftg][:, tb * 512 : (tb + 1) * 512], ps[:], bqt[:, ftg : ftg + 1]
                )

            def emit_k_range(ftg, t0, t1, w):
                ps = mm_p.tile([128, 512], F32, tag="mm")
                for cc in range(8):
                    nc.tensor.matmul(
                        ps[:, 0 : t1 - t0],
                        w[:, cc * 128 : (cc + 1) * 128],
                        xt[cc][:, t0:t1],
                        start=(cc == 0),
                        stop=(cc == 7),
                    )
                nc.vector.tensor_copy(kt[ftg][:, t0:t1], ps[:, 0 : t1 - t0])

            def emit_k_tb(ftg, tb, w):
                emit_k_range(ftg, tb * 512, (tb + 1) * 512, w)

            def emit_v_tt(tt, g):
                # v for head group g (heads 2g, 2g+1) only: spreads the V
                # projection across stripes by per-head-pair av deadlines
                # instead of front-loading all 8 heads into stripes 0-1
                ps = mm_p.tile([128, 512], F32, tag="mm")
                for cc in range(8):
                    nc.tensor.matmul(
                        ps[:, 0:128],
                        xt[cc][:, tt * 128 : (tt + 1) * 128],
                        wv_t[:, cc * FPC + g * 128 : cc * FPC + (g + 1) * 128],
                        start=(cc == 0),
                        stop=(cc == 7),
                    )
                if g == 0:
                    vp_tiles[tt] = vp_p.tile(
                        [128, HPC * (D + 1)], BF16, tag="vp", name=f"vp{tt}"
                    )
                vt = vp_tiles[tt]
                v3 = vt[:].rearrange("p (h e) -> p h e", e=D + 1)
                nc.vector.tensor_copy(v3[:, 2 * g : 2 * g + 2, D], onesc_f[:, 0:2])
                nc.vector.tensor_copy(
                    v3[:, 2 * g : 2 * g + 2, 0:D],
                    ps[:, 0:128].rearrange("p (h d) -> p h d", d=D),
                )

            def emit_scores(st, f, nb, qh, mc, slot):
                # one [128, 1024] psum tile = [head 2f | head 2f+1] for 512
                # queries; the two K=64 matmuls land on PE row tiles (0,0)
                # and (64,0) and run concurrently on hardware.
                ps = sc_p.tile([128, 1024], F32, tag="sc")
                q0 = nb * 1024 + qh * 512
                for j in range(2):
                    off = j * 64
                    nc.tensor.matmul(
                        ps[:, j * 512 : (j + 1) * 512],
                        kt[f][off : off + 64, mc * 128 : (mc + 1) * 128],
                        qt[f][off : off + 64, q0 : q0 + 512],
                        start=True,
                        stop=True,
                    )
                et = et_p.tile([128, 1024], BF16, tag="et", name=f"et{slot}_{mc}")
                if st >= DVE_ST0 and mc in DVE_MC:
                    # offload this tile's exp to the DVE (2-pass cubic^32);
                    # one scratch buf suffices: DVE is strict FIFO, so
                    # pass1(k+1) naturally follows pass2(k)
                    ex = exs_p.tile([128, 1024], F32, tag="exs", name="exscr")
                    nc.vector._custom_dve(
                        _EXP32_CUBIC, out=ex[:], in0=ps[:],
                        s0=SCALE / 32.0, s1=1.0 / 6.0, imm2=0.5,
                    )
                    nc.vector._custom_dve(_POW32, out=et[:], in0=ex[:])
                else:
                    nc.scalar.activation(et[:], ps[:], AF.Exp, scale=SCALE)
                return et

            # the two heads of a pair share one [128q, 128f] ao tile; the
            # j=1 unit transposes both heads at once (one PE transpose + one
            # full-partition DVE copy instead of two half-partition ones)
            ao_pair = {}

            def emit_av(h, nb, qh, q4, ets):
                ftg, j = h // 2, h % 2
                avp = av_p.tile([128, D + 1], F32, tag="av")
                for mc in range(NMC):
                    nc.tensor.matmul(
                        avp[:],
                        ets[mc][:, j * 512 + q4 * 128 : j * 512 + (q4 + 1) * 128],
                        vp_tiles[mc][:, h * (D + 1) : (h + 1) * (D + 1)],
                        start=(mc == 0),
                        stop=(mc == NMC - 1),
                    )
                rc = rc_p.tile([128, 1], F32, tag="rc")
                nc.vector.reciprocal(rc[:], avp[:, D : D + 1])
                if j == 0:
                    ao_pair[0] = ao_p.tile(
                        [128, 2 * D], BF16, tag="ao", name="ao_pair"
                    )
                ao = ao_pair[0]
                nc.vector.tensor_scalar_mul(
                    ao[:, j * D : (j + 1) * D], avp[:, 0:D], rc[:]
                )
                if j == 1:
                    tp = mm_p.tile([128, 512], F32, tag="mm")
                    tpv = tp[:, 0:64].bitcast(BF16)  # [128, 128] bf16 view
                    nc.tensor.transpose(tpv, ao[:], ident[:])
                    col = (nb * 8 + qh * 4 + q4) * 128
                    nc.vector.tensor_copy(attnT[ftg][:, col : col + 128], tpv)

            # token blocks whose fc0/fc1 proj-partial is staged early into
            # dead qt/kt tiles (f32 bitcast views) or the spare ex f32 tiles,
            # bias pre-folded
            exst = [
                ex_p.tile([128, 1024], F32, tag="ex", name=f"exst{i}")
                for i in range(2)
            ]
            # buffer assignment: tt 0-7 (projected at stripe-15-end / tail)
            # stage into the qt/kt tiles that die by stripe 11 plus the ex
            # scratch; tt 8-15 (projected at stripes 14 / 15-early) stage
            # into xt. xt is then fully read by mid-stripe-15, so the NEXT
            # rep's 4MB xt input DMA overlaps this rep's tail instead of
            # serializing behind it.
            stage_map = {0: exst[0], 1: qt[0], 2: kt[0], 3: qt[1],
                         4: kt[1], 5: exst[1], 6: qt[2], 7: kt[2]}
            for j in range(8):
                stage_map[8 + j] = xt[j]
            EX_TTS = (0, 5)  # stage buffers that are natively f32

            def stage_view(tt):
                t = stage_map[tt]
                return t[:] if tt in EX_TTS else t[:].bitcast(F32)

            def emit_proj_stage(tt):
                stv = stage_view(tt)  # [128, 1024] f32 view
                for cb in range(2):
                    ps = mm_p.tile([128, 512], F32, tag="mm")
                    for fc in range(2):
                        nc.tensor.matmul(
                            ps[:],
                            attnT[fc][:, tt * 128 : (tt + 1) * 128],
                            wp_t[:, fc * C + cb * 512 : fc * C + (cb + 1) * 512],
                            start=(fc == 0),
                            stop=(fc == 1),
                        )
                    nc.vector.tensor_add(
                        stv[:, cb * 512 : (cb + 1) * 512],
                        ps[:],
                        bb[:, cb * 512 : (cb + 1) * 512],
                    )

            def emit_proj_tt(tt, staged=False):
                ps = sc_p.tile([128, 1024], F32, tag="sc")
                first = 2 if staged else 0
                for fc in range(first, NFTG):
                    for cb in range(2):
                        nc.tensor.matmul(
                            ps[:, cb * 512 : (cb + 1) * 512],
                            attnT[fc][:, tt * 128 : (tt + 1) * 128],
                            wp_t[:, fc * C + cb * 512 : fc * C + (cb + 1) * 512],
                            start=(fc == first),
                            stop=(fc == NFTG - 1),
                        )
                ot = ot_p.tile([128, C], F32, tag="ot")
                if staged:
                    nc.vector.tensor_add(ot[:], ps[:], stage_view(tt))
                else:
                    nc.vector.tensor_add(ot[:], ps[:], bb[:])
                nc.sync.dma_start(out[tt * 128 : (tt + 1) * 128, :], ot[:])

            # ---- prologue: minimal work before the first scores ----
            wq0 = load_wqk("q", 0)
            wk0 = load_wqk("k", 0)
            dma_xt_tb0()
            nc.sync.dma_start(
                wv_t[:].rearrange("p (a b) -> p a b", b=FPC), wv[:, :, :]
            )
            dma_xt_rest()
            nc.sync.dma_start(
                wp_t[:].rearrange("p (a b) -> p a b", b=C), wp[:, :, :]
            )
            # only the first 128 keys are needed before the first scores;
            # the rest of k tb0 is the first unit of stripe 0
            emit_k_range(0, 0, 128, wk0)
            emit_q_tb(0, 0, wq0)

            # ---- unit schedule: stripe st = f*4 + nb*2 + qh (f = head pair,
            # qh = 512-query half); one unit per mc slot, emitted BEFORE that
            # slot's scores so the PE never blocks on the scores psum buffer
            # with useful work stuck behind it. av units for stripe st run
            # during stripe st+1. ----
            wstate = {}

            def f_dma(kind, ftg):
                def f():
                    wstate[(kind, ftg)] = load_wqk(kind, ftg)
                return f

            def f_qk(kind, ftg, tb):
                emit = emit_q_tb if kind == "q" else emit_k_tb
                def f():
                    w = wstate[(kind, ftg)] if (kind, ftg) in wstate else (
                        wq0 if kind == "q" else wk0
                    )
                    emit(ftg, tb, w)
                return f

            def f_v(tt, g):
                return lambda: emit_v_tt(tt, g)

            def f_av(h, nb, qh, q4, ets):
                return lambda: emit_av(h, nb, qh, q4, ets)

            def avs(prev):
                pf, pnb, pqh, pets = prev
                return [
                    f_av(2 * pf + j, pnb, pqh, q4, pets)
                    for q4 in range(4)
                    for j in range(2)
                ]

            def f_proj(tt):
                return lambda: emit_proj_tt(tt, staged=True)

            def f_stage(tt):
                return lambda: emit_proj_stage(tt)

            prev = None
            for st in range(16):
                f, nb, qh = st // 4, (st // 2) % 2, st % 2
                if f == 3:
                    # run nb=1 before nb=0 for the last head pair so the
                    # tail's av units feed only 4 trailing proj blocks
                    nb = 1 - nb
                if st == 0:
                    units = (
                        [lambda: emit_k_range(0, 128, 512, wk0),
                         f_qk("k", 0, 1), f_qk("k", 0, 2), f_qk("k", 0, 3),
                         f_qk("q", 0, 1)]
                        + [f_v(tt, 0) for tt in range(11)]
                    )
                elif st == 1:
                    units = (
                        [f_v(tt, 0) for tt in range(11, 16)]
                        + avs(prev)
                        + [f_qk("q", 0, 2), f_dma("k", 1), f_qk("k", 1, 0)]
                    )
                else:
                    extra = {
                        2: [f_qk("k", 1, 1), f_qk("k", 1, 2), f_qk("k", 1, 3),
                            f_qk("q", 0, 3), f_dma("q", 1)]
                        + [f_v(tt, 1) for tt in (0, 1, 2)],
                        3: [f_qk("q", 1, 0), f_qk("q", 1, 1)]
                        + [f_v(tt, 1) for tt in (3, 4, 5, 6, 7, 8)],
                        4: [f_dma("k", 2), f_qk("k", 2, 0), f_qk("q", 1, 2)]
                        + [f_v(tt, 1) for tt in (9, 10, 11, 12, 13, 14, 15)],
                        5: [f_qk("k", 2, 1), f_qk("k", 2, 2), f_qk("q", 1, 3)]
                        + [f_v(tt, 2) for tt in (0, 1, 2, 3)],
                        6: [f_qk("k", 2, 3), f_dma("q", 2), f_qk("q", 2, 0),
                            f_qk("q", 2, 1)]
                        + [f_v(tt, 2) for tt in (4, 5, 6, 7)],
                        7: [f_dma("k", 3), f_qk("k", 3, 0), f_qk("k", 3, 1),
                            f_qk("q", 2, 2)]
                        + [f_v(tt, 2) for tt in (8, 9, 10, 11)],
                        8: [f_qk("k", 3, 2), f_qk("k", 3, 3), f_qk("q", 2, 3),
                            f_dma("q", 3)]
                        + [f_v(tt, 2) for tt in (12, 13, 14, 15)],
                        9: [f_qk("q", 3, 2), f_qk("q", 3, 3), f_qk("q", 3, 0),
                            f_qk("q", 3, 1), f_stage(0)]
                        + [f_v(tt, 3) for tt in (0, 1, 2)],
                        10: [f_stage(1), f_stage(2)]
                        + [f_v(tt, 3) for tt in (3, 4, 5, 6, 7)],
                        11: [f_stage(3), f_stage(4)]
                        + [f_v(tt, 3) for tt in (8, 9, 10, 11, 12)],
                        12: [f_stage(5), f_stage(6), f_stage(7)]
                        + [f_v(tt, 3) for tt in (13, 14, 15)],
                        # xt is dead after stripe 12's last v-group units
                        13: [f_stage(tt) for tt in (8, 9, 10)],
                        # proj tt 8-11: attnT[3] cols ready (stripe-12 avs
                        # ran during stripe 13)
                        14: [f_stage(tt) for tt in (11, 12, 13)]
                        + [f_proj(tt) for tt in (8, 9, 10, 11)],
                        # proj 12-15 ready (stripe-13 avs during 14); proj
                        # 0-3 after this stripe's own av units (stripe-14's)
                        15: [f_stage(14), f_stage(15)]
                        + [f_proj(tt) for tt in (12, 13, 14, 15)]
                        + [f_proj(tt) for tt in (0, 1, 2, 3)],
                    }.get(st, [])
                    units = avs(prev) + extra

                ets = []
                for mc in range(NMC):
                    if units:
                        units.pop(0)()
                    ets.append(emit_scores(st, f, nb, qh, mc, slot=st % 3))
                while units:
                    units.pop(0)()
                prev = (f, nb, qh, ets)

            # ---- tail: av of the last stripe (f3, nb0, qh1 -> tt 4..7) ----
            for q4 in range(4):
                emit_av(6, 0, 1, q4, prev[3])
                emit_av(7, 0, 1, q4, prev[3])
                emit_proj_tt(4 + q4, staged=True)

        if reps > 1:
            with tc.For_i(0, reps, 1):
                rep_body()
        else:
            rep_body()

    nc.finalize()
    return nc


_NC_CACHE = {}


def get_nc(reps=1):
    if reps not in _NC_CACHE:
        _NC_CACHE[reps] = build_nc(reps)
    return _NC_CACHE[reps]


def make_in_maps(x, w_qkv, b_qkv, w_proj, b_proj):
    x = np.asarray(x, dtype=np.float32)
    w_qkv = np.asarray(w_qkv, dtype=np.float32)
    b_qkv = np.asarray(b_qkv, dtype=np.float32)
    w_proj = np.asarray(w_proj, dtype=np.float32)
    b_proj = np.asarray(b_proj, dtype=np.float32)

    def pack8(A):  # [1024 cin, 512 f] -> [128, 8, 512]
        return np.ascontiguousarray(
            A.reshape(8, 128, FPC).transpose(1, 0, 2).astype(NPBF)
        )

    ident = np.eye(128, dtype=np.float32).astype(NPBF)
    bp_full = (b_proj + w_proj @ b_qkv[2 * C : 3 * C]).reshape(1, C)

    xT_cache = {}
    in_maps = []
    for c in range(NCORES):
        b, hg = c // 2, c % 2
        F0 = hg * FPC
        if b not in xT_cache:
            xT_cache[b] = np.ascontiguousarray(
                x[b].T.reshape(8, 128, N).transpose(1, 0, 2).astype(NPBF)
            )
        m = {
            "xT": xT_cache[b],
            "wq": pack8(w_qkv[F0 : F0 + FPC, :].T),
            "wk": pack8(w_qkv[C + F0 : C + F0 + FPC, :].T),
            "wv": pack8(w_qkv[2 * C + F0 : 2 * C + F0 + FPC, :].T),
            "wp": np.ascontiguousarray(
                w_proj[:, F0 : F0 + FPC].T.reshape(NFTG, 128, C)
                .transpose(1, 0, 2)
                .astype(NPBF)
            ),
            "bq": np.ascontiguousarray(b_qkv[F0 : F0 + FPC]),
            "bp": np.ascontiguousarray(
                (bp_full if hg == 0 else np.zeros((1, C), np.float32)).astype(NPBF)
            ),
            "ident_in": ident,
        }
        in_maps.append(m)
    return in_maps


def assemble(results):
    y = np.empty((B, N, C), dtype=np.float32)
    for b in range(B):
        np.add(results[2 * b]["out"], results[2 * b + 1]["out"], out=y[b])
    return y


def kernel(x, w_qkv, b_qkv, w_proj, b_proj):
    nc = get_nc()
    in_maps = make_in_maps(x, w_qkv, b_qkv, w_proj, b_proj)
    res = run_bass_kernel_spmd(nc, in_maps, core_ids=list(range(NCORES)))
    return assemble(res.results)


if __name__ == "__main__":
    rng = np.random.default_rng(0)
    x = rng.standard_normal((B, N, C), dtype=np.float32)
    w_qkv = rng.standard_normal((3 * C, C), dtype=np.float32) * C**-0.5
    b_qkv = rng.standard_normal((3 * C,), dtype=np.float32) * 0.02
    w_proj = rng.standard_normal((C, C), dtype=np.float32) * C**-0.5
    b_proj = rng.standard_normal((C,), dtype=np.float32) * 0.02
    y = kernel(x, w_qkv, b_qkv, w_proj, b_proj)
    print("out", y.shape, y.dtype, float(np.abs(y).max()))

